# revision 1
# baseline (speedup 1.0000x reference)
"""Trainium2 Bass kernel for nn_FindNeighbors: row-sharded cosine-sim top-3
neighbor mixing, 8 NeuronCores.

Per core (rows R=2048 of N=16384):
  setup: load emb, 1/norm via exp(-0.5 ln(n2+eps)), prescale->bf16,
         PE-transpose (batched) -> xhatT [2][128, N]
  per 128-row chunk:
    PE    : bf16 matmul -> cos scores (f32 psum, 1024-col pieces)
    ACT   : exp-drain psum -> fp16 exp_scores sbuf + Z accumulation
    DVE   : per-half max8 -> merged top-8 -> one max_index over full row
    SWDGE : indirect-DMA gather of the 8 candidate rows from emb (f32)
    DVE/ACT: f32 rescore (exact cos), top-3, double-softmax weights
    DVE   : weighted sum -> out chunk
fp16 scores only pick candidates; the f32 rescore makes the final top-3 match
the f32 reference (validated vs max_index dup semantics: 0/16384 misses).
All ACT functions (exp/ln/square/copy) live in one act table set -> no swaps.
"""

import sys

sys.path.insert(0, "/opt/trn_rl_repo")

import numpy as np

import concourse.bass as bass
import concourse.tile as tile
from concourse import bacc, mybir
from concourse.bass_utils import run_bass_kernel_spmd
from concourse.masks import make_identity
import concourse.hw_specs as _hw_specs

_orig_get_act_tables = _hw_specs.get_activation_tables


def _patched_get_act_tables(arch):
    tabs = _orig_get_act_tables(arch)
    AFt = mybir.ActivationFunctionType
    for name, funcs in tabs.items():
        has_ln, has_exp = AFt.Ln in funcs, AFt.Exp in funcs
        if has_ln and not has_exp:
            funcs.discard(AFt.Ln)
        if has_exp and not has_ln:
            funcs.discard(AFt.Exp)
    return tabs


import os as _os
if _os.environ.get("NO_TABLE_PATCH", "0") != "1":
    _hw_specs.get_activation_tables = _patched_get_act_tables
    if hasattr(bacc, "get_activation_tables"):
        bacc.get_activation_tables = _patched_get_act_tables

F32 = mybir.dt.float32
BF16 = mybir.dt.bfloat16
FP16 = mybir.dt.float16
U32 = mybir.dt.uint32
AF = mybir.ActivationFunctionType
ALU = mybir.AluOpType

P = 128
D = 256
EPS_BIAS = D * 1e-6  # reference adds 1e-6 per element before the norm sum


def build_nc(N=16384, R=2048, exp_dt=FP16, n_cand=12, batched_gather=False,
             hier_scan=False, SUB=128, KSUB=6):
    """Build the per-core Bass graph. N total rows, R rows for this core."""
    NT = N // P            # emb tiles
    RT = R // P            # row chunks for this core
    DRAIN = 2048 if N % 2048 == 0 else 512
    NPC = N // DRAIN       # drain pieces per chunk
    HALF = N // 2
    TB = 4                 # setup transpose batch (tiles per psum batch)
    TBr = min(TB, RT)      # rows may have fewer tiles (small configs)
    assert NT % TB == 0 and RT % TBr == 0

    nc = bacc.Bacc(None, target_bir_lowering=False, debug=False)
    emb = nc.dram_tensor("emb", [N, D], F32, kind="ExternalInput")
    rows = nc.dram_tensor("rows", [R, D], F32, kind="ExternalInput")
    out = nc.dram_tensor("out", [R, D], F32, kind="ExternalOutput")

    with tile.TileContext(nc) as tc:
        with (
            tc.tile_pool(name="persist", bufs=1) as persist,
            tc.tile_pool(name="stage", bufs=2) as stage,
            tc.tile_pool(name="expbuf", bufs=2) as expbuf,
            tc.tile_pool(name="cands", bufs=2) as cands,
            tc.tile_pool(name="small", bufs=4) as small,
            tc.tile_pool(name="dram", bufs=2, space="DRAM") as drampool,
            tc.tile_pool(name="psum_mm", bufs=2, space="PSUM") as psum_mm,
        ):
            ident = persist.tile([P, P], BF16)
            make_identity(nc, ident[:])
            epsb = persist.tile([P, 1], F32, name="epsb")
            nc.vector.memset(epsb[:], float(EPS_BIAS))
            NS = N // SUB
            if hier_scan:
                pio_d = nc.inline_tensor(
                    (np.arange(P, dtype=np.int32) * NS).reshape(P, 1),
                    name="pio_const")
                pio = persist.tile([P, 1], mybir.dt.int32, name="pio")
                nc.sync.dma_start(pio[:], pio_d[:])
                pbasef = persist.tile([P, 1], F32, name="pbasef")
                nc.vector.tensor_copy(pbasef[:], pio[:])

            # persistent transposed, prescaled bf16 matrices
            xhatT = [persist.tile([P, N], BF16, tag=f"xhatT{h}", name=f"xhatT{h}")
                     for h in range(2)]
            rowsT = [persist.tile([P, R], BF16, tag=f"rowsT{h}", name=f"rowsT{h}")
                     for h in range(2)]
            rows_nat = persist.tile([P, RT * D], F32)   # natural-layout rows
            rn_rows = persist.tile([P, RT], F32)        # 1/norm per query row

            def prep_batch(src_dram, b, destT, dest_nat=None, dest_rn=None,
                           TB=TB):
                """Load TB tiles, 1/norms, prescale->bf16, transpose (batched)."""
                if dest_nat is None:
                    etile = stage.tile([P, TB * D], F32, tag="etile",
                                       name="etile")
                    nc.sync.dma_start(
                        etile[:].rearrange("p (t d) -> p t d", t=TB),
                        src_dram[b * TB * P:(b + 1) * TB * P, :].rearrange(
                            "(t p) d -> p t d", p=P))
                    ets = [etile[:, k * D:(k + 1) * D] for k in range(TB)]
                else:
                    ets = [dest_nat[:, (b * TB + k) * D:(b * TB + k + 1) * D]
                           for k in range(TB)]
                n2b = stage.tile([P, TB], F32, tag="n2b", name="n2b")
                sq = stage.tile([P, D], F32, tag="sq", name="sq")
                for k in range(TB):
                    nc.scalar.activation(sq[:], ets[k], AF.Square,
                                         accum_out=n2b[:, k:k + 1])
                # rn = exp(-0.5 ln(n2+eps)); ln+exp live in one table set
                # (the hw_specs patch above stops walrus picking ln-only sets)
                nrb = stage.tile([P, TB], F32, tag="nrb", name="nrb")
                nc.scalar.activation(nrb[:], n2b[:], AF.Ln, bias=epsb[:])
                rnb = (stage.tile([P, TB], F32, tag="rnb", name="rnb")
                       if dest_rn is None else dest_rn[:, b * TB:(b + 1) * TB])
                nc.scalar.activation(rnb, nrb[:], AF.Exp, scale=-0.5)
                psf = psum_mm.tile([P, DRAIN], F32, tag="ps_mm", name="ps_tr")
                ps = psf[:].bitcast(BF16)
                for k in range(TB):
                    xh = stage.tile([P, D], BF16, tag="xh", name="xh")
                    nc.vector.tensor_scalar_mul(xh[:], ets[k],
                                                rnb[:, k:k + 1])
                    for h in range(2):
                        cpos = (k * 2 + h) * P
                        nc.tensor.transpose(ps[:, cpos:cpos + P],
                                            xh[:, h * P:(h + 1) * P], ident[:])
                for h in range(2):
                    srcv = ps[:, 0:TB * 2 * P].rearrange(
                        "p (k h f) -> p k (h f)", k=TB, h=2)
                    srcv = srcv[:, :, h * P:(h + 1) * P]
                    dst = destT[h][:, b * TB * P:(b + 1) * TB * P].rearrange(
                        "p (k f) -> p k f", k=TB)
                    nc.vector.tensor_copy(dst, srcv)

            for b in range(NT // TB):
                prep_batch(emb, b, xhatT)
            nc.sync.dma_start(
                rows_nat[:].rearrange("p (t d) -> p t d", t=RT),
                rows[:].rearrange("(t p) d -> p t d", p=P))
            for b in range(RT // TBr):
                prep_batch(rows, b, rowsT, dest_nat=rows_nat, dest_rn=rn_rows,
                           TB=TBr)

            # main loop over row chunks
            for c in range(RT):
                lhs = [rowsT[h][:, c * P:(c + 1) * P] for h in range(2)]
                zparts = small.tile([P, NPC], F32, tag="zparts")
                expf = expbuf.tile([P, N], exp_dt, tag="expf")
                for pc in range(NPC):
                    ps = psum_mm.tile([P, DRAIN], F32, name="ps_mm")
                    for s in range(DRAIN // 512):
                        col0 = pc * DRAIN + s * 512
                        nc.tensor.matmul(
                            ps[:, s * 512:(s + 1) * 512], lhs[0],
                            xhatT[0][:, col0:col0 + 512],
                            start=True, stop=False,
                        )
                        nc.tensor.matmul(
                            ps[:, s * 512:(s + 1) * 512], lhs[1],
                            xhatT[1][:, col0:col0 + 512],
                            start=False, stop=True,
                        )
                    nc.scalar.activation(
                        expf[:, pc * DRAIN:(pc + 1) * DRAIN], ps[:],
                        AF.Exp, accum_out=zparts[:, pc:pc + 1],
                    )
                # top-8 candidates via subtile hierarchy:
                # fp16 subtile maxes (2x DVE mode), top-KSUB subtiles,
                # bounce expf to DRAM, per-row SWDGE gather of the winning
                # subtiles, short max8/max_index scan, index reassembly.
                i8 = small.tile([P, n_cand], U32, tag="i8")
                if hier_scan:
                    dtile = drampool.tile([P * NS, SUB], exp_dt, tag="dt",
                                          name="dt")
                    dv = dtile[:].rearrange("(p s) f -> p (s f)", p=P)
                    NDMA = 8
                    W = N // NDMA
                    for i in range(NDMA):
                        nc.sync.dma_start(dv[:, i * W:(i + 1) * W],
                                          expf[:, i * W:(i + 1) * W])
                    # subtile maxes via a fold tree of tensor_tensor max
                    # (2x DVE mode on fp16; tensor_reduce/max8 are 1x-locked)
                    folda = small.tile([P, N // 2], exp_dt, tag="folda",
                                       bufs=1)
                    ev = expf[:].rearrange("p (s f) -> p s f", s=NS)
                    fw = SUB // 2
                    av = folda[:].rearrange("p (s f) -> p s f", s=NS)
                    nc.vector.tensor_tensor(
                        av[:, :, 0:fw], ev[:, :, 0:fw], ev[:, :, fw:SUB],
                        op=ALU.max)
                    fw //= 2
                    while fw >= 1:
                        dstv = (av[:, :, 0:fw] if fw > 1
                                else None)
                        if fw > 1:
                            nc.vector.tensor_tensor(
                                av[:, :, 0:fw], av[:, :, 0:fw],
                                av[:, :, fw:2 * fw], op=ALU.max)
                        fw //= 2
                    subm = small.tile([P, NS], exp_dt, tag="subm")
                    nc.vector.tensor_tensor(
                        subm[:].rearrange("p (s f) -> p s f", s=NS),
                        av[:, :, 0:1], av[:, :, 1:2], op=ALU.max)
                    v8s = small.tile([P, 8], exp_dt, tag="v8s")
                    nc.vector.max(v8s[:], subm[:])
                    i8s = small.tile([P, 8], U32, tag="i8s")
                    nc.vector.max_index(i8s[:], v8s[:], subm[:])
                    goff = small.tile([P, KSUB], U32, tag="goff")
                    nc.vector.tensor_scalar(goff[:], i8s[:, 0:KSUB],
                                            pbasef[:, 0:1], None, op0=ALU.add)
                    gsub = small.tile([P, KSUB * SUB], exp_dt, tag="gsub")
                    nc.gpsimd.indirect_dma_start(
                        out=gsub[:].rearrange("p (j f) -> p j f", j=KSUB),
                        out_offset=None, in_=dtile[:],
                        in_offset=bass.IndirectOffsetOnAxis(ap=goff[:], axis=0))
                    v8 = small.tile([P, 8], exp_dt, tag="v8")
                    nc.vector.max(v8[:], gsub[:])
                    pos = small.tile([P, 8], U32, tag="pos")
                    nc.vector.max_index(pos[:], v8[:], gsub[:])
                    slot = small.tile([P, 8], U32, tag="slot")
                    nc.vector.tensor_scalar(slot[:], pos[:],
                                            SUB.bit_length() - 1, None,
                                            op0=ALU.logical_shift_right)
                    loc = small.tile([P, 8], U32, tag="loc")
                    nc.vector.tensor_scalar(loc[:], pos[:], SUB - 1, None,
                                            op0=ALU.bitwise_and)
                    i8sf = small.tile([P, 8], F32, tag="i8sf")
                    nc.vector.tensor_copy(i8sf[:], i8s[:])
                    subsel = small.tile([P, 8], U32, tag="subsel")
                    msk = small.tile([P, 8], U32, tag="msk")
                    for k in range(KSUB):
                        nc.vector.tensor_scalar(msk[:], slot[:], k,
                                                None, op0=ALU.is_equal)
                        if k == 0:
                            nc.vector.tensor_scalar_mul(subsel[:], msk[:],
                                                        i8sf[:, 0:1])
                        else:
                            nc.vector.scalar_tensor_tensor(
                                out=subsel[:], in0=msk[:],
                                scalar=i8sf[:, k:k + 1], in1=subsel[:],
                                op0=ALU.mult, op1=ALU.add)
                    nc.vector.tensor_scalar(i8[:], subsel[:], SUB,
                                            None, op0=ALU.mult)
                    nc.vector.tensor_add(i8[:], i8[:], loc[:])
                else:
                    # halves-pair scan: fold the row in place (2x DVE mode on
                    # fp16 tensor_tensor), scan the 8192-wide fold, then take
                    # BOTH members {p, p+HALF} of each top-8 pair as the 16
                    # rescore candidates. Pair-level containment is exact and
                    # fp16-tie-safe (validated: 0/16384 misses).
                    nc.vector.tensor_tensor(expf[:, 0:HALF], expf[:, 0:HALF],
                                            expf[:, HALF:N], op=ALU.max)
                    v8 = small.tile([P, 8], exp_dt, tag="v8")
                    nc.vector.max(v8[:], expf[:, 0:HALF])
                    pos = small.tile([P, 8], U32, tag="pos")
                    nc.vector.max_index(pos[:], v8[:], expf[:, 0:HALF])
                    NPAIR = n_cand // 2
                    nc.vector.tensor_copy(i8[:, 0:NPAIR], pos[:, 0:NPAIR])
                    nc.vector.tensor_scalar_add(i8[:, NPAIR:n_cand],
                                                pos[:, 0:NPAIR], HALF)

                z = small.tile([P, 1], F32, tag="z")
                nc.vector.tensor_reduce(z[:], zparts[:], mybir.AxisListType.X,
                                        ALU.add)
                rz = small.tile([P, 1], F32, tag="rz")
                nc.vector.reciprocal(rz[:], z[:])

                # gather the candidate rows (unscaled f32) from DRAM
                cand = cands.tile([P, n_cand * D], F32, tag="cand")
                if batched_gather:
                    nc.gpsimd.indirect_dma_start(
                        out=cand[:].rearrange("p (j d) -> p j d", j=n_cand),
                        out_offset=None,
                        in_=emb[:],
                        in_offset=bass.IndirectOffsetOnAxis(ap=i8[:], axis=0),
                    )
                else:
                    for j in range(n_cand):
                        nc.gpsimd.indirect_dma_start(
                            out=cand[:, j * D:(j + 1) * D],
                            out_offset=None,
                            in_=emb[:],
                            in_offset=bass.IndirectOffsetOnAxis(
                                ap=i8[:, j:j + 1], axis=0
                            ),
                        )

                # f32 rescore: cos = <q, cand> * rn_q * rn_c
                n2c = small.tile([P, n_cand], F32, tag="n2c")
                sdot = small.tile([P, n_cand], F32, tag="sdot")
                scr = cands.tile([P, D], F32, tag="scr", bufs=1)
                scr2 = cands.tile([P, D], F32, tag="scr2", bufs=1)
                qrow = rows_nat[:, c * D:(c + 1) * D]
                for j in range(n_cand):
                    cj = cand[:, j * D:(j + 1) * D]
                    nc.scalar.activation(scr[:], cj, AF.Square,
                                         accum_out=n2c[:, j:j + 1])
                    nc.vector.scalar_tensor_tensor(
                        out=scr2[:], in0=cj, scalar=1.0, in1=qrow,
                        op0=ALU.mult, op1=ALU.mult, accum_out=sdot[:, j:j + 1],
                    )
                # rn_c = exp(-0.5 ln(n2c+eps)) on ACT (one table set)
                lnc = small.tile([P, n_cand], F32, tag="lnc")
                nc.scalar.activation(lnc[:], n2c[:], AF.Ln, bias=epsb[:])
                rnc = small.tile([P, n_cand], F32, tag="rnc")
                nc.scalar.activation(rnc[:], lnc[:], AF.Exp, scale=-0.5)
                cos8 = small.tile([P, n_cand], F32, tag="cos8")
                nc.vector.scalar_tensor_tensor(
                    out=cos8[:], in0=sdot[:], scalar=rn_rows[:, c:c + 1],
                    in1=rnc[:], op0=ALU.mult, op1=ALU.mult)

                # top-3 of the rescored cos, second softmax weights
                v3 = small.tile([P, 8], F32, tag="v3")
                nc.vector.max(v3[:], cos8[:])
                m3 = small.tile([P, n_cand], F32, tag="m3")
                nc.vector.tensor_scalar(
                    m3[:], cos8[:], v3[:, 2:3], None, op0=ALU.is_ge
                )
                e8 = small.tile([P, n_cand], F32, tag="e8")
                nc.scalar.activation(e8[:], cos8[:], AF.Exp)
                e2 = small.tile([P, n_cand], F32, tag="e2")
                nc.scalar.activation(e2[:], e8[:], AF.Exp, scale=rz[:])
                e2m = small.tile([P, n_cand], F32, tag="e2m")
                s3 = small.tile([P, 1], F32, tag="s3")
                nc.vector.scalar_tensor_tensor(
                    out=e2m[:], in0=e2[:], scalar=1.0, in1=m3[:],
                    op0=ALU.mult, op1=ALU.mult, accum_out=s3[:])
                rs3 = small.tile([P, 1], F32, tag="rs3")
                nc.vector.reciprocal(rs3[:], s3[:])
                w8 = small.tile([P, n_cand], F32, tag="w8")
                nc.vector.tensor_scalar_mul(w8[:], e2m[:], rs3[:])

                # weighted sum of candidate rows
                acc = cands.tile([P, D], F32, tag="acc")
                nc.vector.tensor_scalar_mul(acc[:], cand[:, 0:D], w8[:, 0:1])
                for j in range(1, n_cand):
                    nc.vector.scalar_tensor_tensor(
                        out=acc[:], in0=cand[:, j * D:(j + 1) * D],
                        scalar=w8[:, j:j + 1], in1=acc[:],
                        op0=ALU.mult, op1=ALU.add,
                    )
                nc.sync.dma_start(out[c * P:(c + 1) * P, :], acc[:])

    nc.compile()
    return nc


_NC_CACHE = {}


def _get_nc(key=(16384, 2048)):
    if key not in _NC_CACHE:
        _NC_CACHE[key] = build_nc(*key)
    return _NC_CACHE[key]


def kernel(sess_emb: np.ndarray) -> np.ndarray:
    N, D_ = sess_emb.shape
    assert (N, D_) == (16384, 256)
    n_cores = 8
    R = N // n_cores
    sess_emb = np.ascontiguousarray(sess_emb, dtype=np.float32)
    nc = _get_nc((N, R))
    in_maps = [
        {"emb": sess_emb, "rows": sess_emb[i * R:(i + 1) * R]}
        for i in range(n_cores)
    ]
    res = run_bass_kernel_spmd(nc, in_maps, core_ids=list(range(n_cores)))
    return np.concatenate([r["out"] for r in res.results], axis=0)


if __name__ == "__main__":
    x = np.random.randn(16384, 256).astype(np.float32)
    y = kernel(x)
    print(y.shape, y.dtype)

